# revision 14
# baseline (speedup 1.0000x reference)
"""Trainium2 Bass/Tile kernel for nn_DecoderRNN (GRU decoder, teacher forcing,
vocab projection + log_softmax) on 8 NeuronCores.

Sharding:
  - GRU recurrence: hidden dim sharded 8-way (core j owns h columns
    [128j, 128j+128)); per-step AllGather (bf16, 16KB/rank) rebuilds the full
    hidden state on every core in transposed layout, which is exactly the
    stationary operand the output projection needs.
  - Output projection + log_softmax: vocab sharded 8-way (core j owns
    out_W rows [4000j, 4000j+4000)), interleaved step-by-step with the
    recurrence so the PE stays busy during collective latency.
  - One final AllReduce combines per-core sum(exp(logits)) into the global
    log_softmax denominator.

Layout conventions (device):
  rows r = t*64 + b  (t-major) for the [T*B, ...] row dims.
  "T-layout" SBUF tensors keep H on partitions as 8 k-tiles side by side:
     x.T, hs.T, W.T tiles are [128p, k, cols].
"""

import sys

sys.path.insert(0, "/opt/trn_rl_repo")

import numpy as np
import ml_dtypes

import concourse.bass as bass
import concourse.tile as tile
from concourse import bacc
from concourse import mybir
from concourse.bass import IndirectOffsetOnAxis
from concourse.masks import make_identity

AF = mybir.ActivationFunctionType
ALU = mybir.AluOpType
DT = mybir.dt
BF16 = ml_dtypes.bfloat16

B, S, H, V, T = 64, 32, 1024, 32000, 32
NC = 8
VL = V // NC  # 4000
KT = H // 128  # 8
R = T * B  # 2048
RG = [list(range(NC))]
HALF_W = (2048, 1952)  # projection half-span widths (4*512, 3*512+416)


def _v(t, k):
    """View a [128, k*c] tile as [128, k, c]."""
    return t[:].rearrange("p (k c) -> p k c", k=k)


def build_nc():
    nc = bacc.Bacc(
        "TRN2",
        target_bir_lowering=False,
        debug=False,
        num_devices=NC,
    )

    # ---------------- I/O ----------------
    tok_d = nc.dram_tensor("tok", [128, 16], DT.int32, kind="ExternalInput")
    emb_d = nc.dram_tensor("emb", [V, H], DT.float32, kind="ExternalInput")
    h0T_d = nc.dram_tensor("h0T", [128, KT * 128], DT.bfloat16, kind="ExternalInput")
    h0sh_d = nc.dram_tensor("h0sh", [128, 64], DT.float32, kind="ExternalInput")
    whh_d = nc.dram_tensor("whh", [128, KT * 384], DT.bfloat16, kind="ExternalInput")
    whl_d = nc.dram_tensor("whl", [128, KT * 384], DT.bfloat16, kind="ExternalInput")
    wih_d = nc.dram_tensor("wih", [128, KT * 384], DT.float32, kind="ExternalInput")
    bhh_d = nc.dram_tensor("bhh", [128, 3], DT.float32, kind="ExternalInput")
    bih_d = nc.dram_tensor("bih", [128, 3], DT.float32, kind="ExternalInput")
    woT_d = nc.dram_tensor("woT", [128, KT * VL], DT.bfloat16, kind="ExternalInput")
    outb_d = nc.dram_tensor("outb", [128, VL], DT.bfloat16, kind="ExternalInput")

    outp_d = nc.dram_tensor("outp", [B, T, VL], DT.float32, kind="ExternalOutput")
    hfin_d = nc.dram_tensor("hfin", [H, B], DT.float32, kind="ExternalOutput")

    spill_d = nc.dram_tensor("spill", [16, 128, VL], DT.bfloat16)  # internal

    with tile.TileContext(nc) as tc:
        const = tc.alloc_tile_pool(name="const", bufs=1)
        wk = tc.alloc_tile_pool(name="wk", bufs=2)
        gt = tc.alloc_tile_pool(name="gt", bufs=2)
        spn = tc.alloc_tile_pool(name="spn", bufs=3)
        ps_a = tc.alloc_tile_pool(name="ps_a", bufs=2, space="PSUM")
        ps_gh = tc.alloc_tile_pool(name="ps_gh", bufs=2, space="PSUM")
        ps_pr = tc.alloc_tile_pool(name="ps_pr", bufs=1, space="PSUM")
        hsm = tc.alloc_tile_pool(name="hsm", bufs=4)
        dram = tc.alloc_tile_pool(name="dram", bufs=3, space="DRAM")

        # ------------- persistent SBUF -------------
        woT = const.tile([128, KT * VL], DT.bfloat16)
        whh = const.tile([128, KT * 384], DT.bfloat16)
        whl = const.tile([128, KT * 384], DT.bfloat16)
        wih = const.tile([128, KT * 384], DT.float32)
        h0T = const.tile([128, KT * 128], DT.bfloat16)
        gi = const.tile([128, 3 * R], DT.float32)
        outb = const.tile([128, VL], DT.bfloat16)
        bhh = const.tile([128, 3], DT.float32)
        bih = const.tile([128, 3], DT.float32)
        tok = const.tile([128, 16], DT.int32)
        acc = const.tile([128, 32], DT.float32)
        sloc = const.tile([128, 16], DT.float32)
        sglob = const.tile([128, 16], DT.float32)
        lse = const.tile([128, 16], DT.float32)
        nlse = const.tile([128, 16], DT.float32)
        h0sh = const.tile([128, 64], DT.float32)
        ident = const.tile([128, 128], DT.float32)

        # ------------- load constants -------------
        nc.sync.dma_start(out=woT[:], in_=woT_d[:, :])
        nc.sync.dma_start(out=whh[:], in_=whh_d[:, :])
        nc.sync.dma_start(out=whl[:], in_=whl_d[:, :])
        nc.sync.dma_start(out=wih[:], in_=wih_d[:, :])
        nc.sync.dma_start(out=outb[:], in_=outb_d[:, :])
        nc.sync.dma_start(out=bhh[:], in_=bhh_d[:, :])
        nc.sync.dma_start(out=bih[:], in_=bih_d[:, :])
        nc.sync.dma_start(out=tok[:], in_=tok_d[:, :])
        nc.sync.dma_start(out=h0T[:], in_=h0T_d[:, :])
        nc.sync.dma_start(out=h0sh[:], in_=h0sh_d[:, :])
        make_identity(nc, ident[:])

        whh_v = _v(whh, KT)  # [128, k, 384]
        whl_v = _v(whl, KT)
        wih_v = _v(wih, KT)
        woT_v = _v(woT, KT)  # [128, k, 4000]
        h0T_v = _v(h0T, KT)  # [128, k, 128] (hi|lo)
        gi_v = _v(gi, 3)  # [128, g, 2048]

        # x.T chunk tiles: [128, k, 512] covering rows 512g..512g+512
        xT_tiles = {}

        def emit_rblock(i):
            """Gather+relu+transpose 128 embedding rows into xT chunk g=i//4."""
            g, rl = i // 4, (i % 4) * 128
            if g not in xT_tiles:
                xT_tiles[g] = wk.tile(
                    [128, KT * 512], DT.float32, name=f"xT{g}", tag="xT", bufs=1
                )
            xch = _v(xT_tiles[g], KT)
            xg = wk.tile([128, H], DT.float32, tag="xg")
            nc.gpsimd.indirect_dma_start(
                out=xg[:],
                out_offset=None,
                in_=emb_d[:, :],
                in_offset=IndirectOffsetOnAxis(ap=tok[:, i : i + 1], axis=0),
            )
            for k in range(KT):
                pt = ps_a.tile([128, 128], DT.float32, tag="ps_a")
                nc.tensor.transpose(
                    out=pt[:, 0:128], in_=xg[:, 128 * k : 128 * (k + 1)], identity=ident[:]
                )
                # fused relu: x.T = max(x.T, 0)
                nc.vector.tensor_scalar(
                    out=xch[:, k, rl : rl + 128],
                    in0=pt[:, 0:128],
                    scalar1=0.0,
                    scalar2=None,
                    op0=ALU.max,
                )

        def emit_gi_chunk(g):
            """gi[:, :, 512g:512g+512] = W_ih_j @ x.T chunk + b_ih."""
            xch = _v(xT_tiles[g], KT)
            for m in range(3):
                ps = ps_a.tile([128, 512], DT.float32, tag="ps_a")
                for k in range(KT):
                    nc.tensor.matmul(
                        ps[:, :],
                        wih_v[:, k, 128 * m : 128 * (m + 1)],
                        xch[:, k, :],
                        start=(k == 0),
                        stop=(k == KT - 1),
                    )
                nc.scalar.activation(
                    out=gi_v[:, m, 512 * g : 512 * (g + 1)],
                    in_=ps[:, :],
                    func=AF.Identity,
                    bias=bih[:, m : m + 1],
                )

        hs_tiles = {}

        def emit_proj_half(mi, hf):
            """Projection for row-tile mi (rows 128mi..), vocab half hf."""
            W = HALF_W[hf]
            hsv = _v(hs_tiles[mi], KT)
            pp = ps_pr.tile([128, 2048], DT.float32, tag="ps_pr")
            for nl in range(4):
                n = 4 * hf + nl
                w = 512 if n < 7 else 416
                for k in range(KT):
                    nc.tensor.matmul(
                        pp[:, 512 * nl : 512 * nl + w],
                        hsv[:, k, 0:128],
                        woT_v[:, k, 512 * n : 512 * n + w],
                        start=(k == 0),
                        stop=(k == KT - 1),
                    )
            sp = spn.tile([128, 2048], DT.bfloat16, tag="span", bufs=2)
            nc.vector.tensor_tensor(
                out=sp[:, :W],
                in0=pp[:, :W],
                in1=outb[:, 2048 * hf : 2048 * hf + W],
                op=ALU.add,
            )
            ex = spn.tile([128, 2048], DT.bfloat16, tag="expo", bufs=1)
            nc.scalar.activation(
                out=ex[:, :W],
                in_=sp[:, :W],
                func=AF.Exp,
                accum_out=acc[:, 2 * mi + hf : 2 * mi + hf + 1],
            )
            nc.sync.dma_start(
                out=spill_d[mi, :, 2048 * hf : 2048 * hf + W], in_=sp[:, :W]
            )

        # ------------- prologue: first 4 row-blocks + gi chunk 0 -------------
        for i in range(4):
            emit_rblock(i)
        emit_gi_chunk(0)

        # ------------- recurrence + interleaved projection -------------
        h_prev = h0sh  # [128, 64] f32, this core's h-column shard
        rhs_v = h0T_v  # [128, k, 64] bf16 view holding h_{t-1}.T
        rhs_off = 0

        for t in range(T):
            # gh.T shard: 3 gate tiles of [128, 64], one PSUM bank
            gh = ps_gh.tile([128, 192], DT.float32, tag="ps_gh")
            for g in range(3):
                for k in range(KT):
                    lhs_hi = whh_v[:, k, 128 * g : 128 * (g + 1)]
                    lhs_lo = whl_v[:, k, 128 * g : 128 * (g + 1)]
                    r_hi = rhs_v[:, k, rhs_off : rhs_off + 64]
                    r_lo = rhs_v[:, k, rhs_off + 64 : rhs_off + 128]
                    out_g = gh[:, 64 * g : 64 * (g + 1)]
                    nc.tensor.matmul(out_g, lhs_hi, r_hi, start=(k == 0), stop=False)
                    nc.tensor.matmul(out_g, lhs_hi, r_lo, start=False, stop=False)
                    nc.tensor.matmul(
                        out_g, lhs_lo, r_hi, start=False, stop=(k == KT - 1)
                    )

            # gates: r = sigma(gh_r + gi_r + bhh_r), z likewise,
            # n = tanh(gi_n + r*(gh_n + bhh_n)), h' = n + z*(h - n)
            # sigma(x) = 1/(1+exp(-x)); tanh(x) = 2/(1+exp(-2x)) - 1
            tr = gt.tile([128, 64], DT.float32, tag="tr")
            nc.vector.tensor_tensor(
                out=tr[:], in0=gh[:, 0:64], in1=gi_v[:, 0, 64 * t : 64 * t + 64], op=ALU.add
            )
            er = gt.tile([128, 64], DT.float32, tag="er")
            nc.scalar.activation(
                out=er[:], in_=tr[:], func=AF.Exp, scale=-1.0, bias=bhh[:, 0:1]
            )
            dr = gt.tile([128, 64], DT.float32, tag="dr")
            nc.vector.tensor_scalar(
                out=dr[:], in0=er[:], scalar1=1.0, scalar2=None, op0=ALU.add
            )
            rr = gt.tile([128, 64], DT.float32, tag="rr")
            nc.vector.reciprocal(out=rr[:], in_=dr[:])

            tz = gt.tile([128, 64], DT.float32, tag="tz")
            nc.vector.tensor_tensor(
                out=tz[:], in0=gh[:, 64:128], in1=gi_v[:, 1, 64 * t : 64 * t + 64], op=ALU.add
            )
            ez = gt.tile([128, 64], DT.float32, tag="ez")
            nc.scalar.activation(
                out=ez[:], in_=tz[:], func=AF.Exp, scale=-1.0, bias=bhh[:, 1:2]
            )
            dz = gt.tile([128, 64], DT.float32, tag="dz")
            nc.vector.tensor_scalar(
                out=dz[:], in0=ez[:], scalar1=1.0, scalar2=None, op0=ALU.add
            )
            zz = gt.tile([128, 64], DT.float32, tag="zz")
            nc.vector.reciprocal(out=zz[:], in_=dz[:])

            hn = gt.tile([128, 64], DT.float32, tag="hn")
            nc.scalar.activation(
                out=hn[:], in_=gh[:, 128:192], func=AF.Identity, bias=bhh[:, 2:3]
            )
            uu = gt.tile([128, 64], DT.float32, tag="uu")
            nc.vector.tensor_tensor(out=uu[:], in0=rr[:], in1=hn[:], op=ALU.mult)
            ww = gt.tile([128, 64], DT.float32, tag="ww")
            nc.vector.tensor_tensor(
                out=ww[:], in0=uu[:], in1=gi_v[:, 2, 64 * t : 64 * t + 64], op=ALU.add
            )
            en = gt.tile([128, 64], DT.float32, tag="en")
            nc.scalar.activation(out=en[:], in_=ww[:], func=AF.Exp, scale=-2.0)
            dn = gt.tile([128, 64], DT.float32, tag="dn")
            nc.vector.tensor_scalar(
                out=dn[:], in0=en[:], scalar1=1.0, scalar2=None, op0=ALU.add
            )
            pn = gt.tile([128, 64], DT.float32, tag="pn")
            nc.vector.reciprocal(out=pn[:], in_=dn[:])
            nn = gt.tile([128, 64], DT.float32, tag="nn")
            nc.vector.tensor_scalar(
                out=nn[:], in0=pn[:], scalar1=2.0, scalar2=-1.0, op0=ALU.mult, op1=ALU.add
            )

            d2 = gt.tile([128, 64], DT.float32, tag="d2")
            nc.vector.tensor_tensor(out=d2[:], in0=h_prev[:], in1=nn[:], op=ALU.subtract)
            qq = gt.tile([128, 64], DT.float32, tag="qq")
            nc.vector.tensor_tensor(out=qq[:], in0=zz[:], in1=d2[:], op=ALU.mult)
            hnew = gt.tile([128, 64], DT.float32, tag="hnew")
            nc.vector.tensor_tensor(out=hnew[:], in0=nn[:], in1=qq[:], op=ALU.add)
            hnb = gt.tile([128, 128], DT.bfloat16, tag="hnb")
            nc.vector.tensor_copy(out=hnb[:, 0:64], in_=hnew[:])
            hlo = gt.tile([128, 64], DT.float32, tag="hlo")
            nc.vector.tensor_tensor(
                out=hlo[:], in0=hnew[:], in1=hnb[:, 0:64], op=ALU.subtract
            )
            nc.vector.tensor_copy(out=hnb[:, 64:128], in_=hlo[:])

            # AllGather (h'.T hi | lo) across the 8 cores
            agi = dram.tile([128, 128], DT.bfloat16, tag="agi")
            nc.sync.dma_start(out=agi[:], in_=hnb[:])
            ago = dram.tile([KT * 128, 128], DT.bfloat16, tag="ago")
            nc.gpsimd.collective_compute(
                "AllGather",
                ALU.bypass,
                replica_groups=RG,
                ins=[agi[:].opt()],
                outs=[ago[:].opt()],
            )
            ago_v = ago[:].rearrange("(k p) c -> p k c", p=128)
            mi_t, par = t // 2, t % 2
            if par == 0:
                hs_tiles[mi_t] = hsm.tile(
                    [128, KT * 128], DT.bfloat16, name=f"hs{mi_t}", tag="hsm"
                )
            hsv_t = _v(hs_tiles[mi_t], KT)
            nc.sync.dma_start(
                out=hsv_t[:, :, 64 * par : 64 * par + 64], in_=ago_v[:, :, 0:64]
            )
            hcur = gt.tile([128, KT * 128], DT.bfloat16, tag="hcur")
            nc.sync.dma_start(out=_v(hcur, KT), in_=ago_v)

            h_prev = hnew
            rhs_v = _v(hcur, KT)
            rhs_off = 0

            # trailing prologue work (embedding/x.T/gi for later steps)
            if t < 12:
                emit_rblock(4 + t)
            if t in (3, 7, 11):
                emit_gi_chunk(t // 4 + 1)
            # interleaved projection halves
            if t >= 2:
                emit_proj_half((t - 2) // 2, (t - 2) % 2)

        # final hidden state (exact f32 shards)
        agfi = dram.tile([128, 64], DT.float32, tag="agfi")
        nc.sync.dma_start(out=agfi[:], in_=h_prev[:])
        agfo = dram.tile([KT * 128, 64], DT.float32, tag="agfo")
        nc.gpsimd.collective_compute(
            "AllGather",
            ALU.bypass,
            replica_groups=RG,
            ins=[agfi[:].opt()],
            outs=[agfo[:].opt()],
        )
        nc.gpsimd.dma_start(out=hfin_d[:, :], in_=agfo[:])

        # remaining projection halves
        emit_proj_half(15, 0)
        emit_proj_half(15, 1)

        # ------------- lse: AllReduce of sum(exp(logits)) -------------
        for mi in range(16):
            nc.vector.tensor_tensor(
                out=sloc[:, mi : mi + 1],
                in0=acc[:, 2 * mi : 2 * mi + 1],
                in1=acc[:, 2 * mi + 1 : 2 * mi + 2],
                op=ALU.add,
            )
        ar_i = dram.tile([128, 16], DT.float32, tag="ar_i")
        nc.sync.dma_start(out=ar_i[:], in_=sloc[:])
        ar_o = dram.tile([128, 16], DT.float32, tag="ar_o")
        nc.gpsimd.collective_compute(
            "AllReduce",
            ALU.add,
            replica_groups=RG,
            ins=[ar_i[:].opt()],
            outs=[ar_o[:].opt()],
        )
        nc.sync.dma_start(out=sglob[:], in_=ar_o[:])
        nc.scalar.activation(out=lse[:], in_=sglob[:], func=AF.Ln)
        nc.vector.tensor_scalar(
            out=nlse[:], in0=lse[:], scalar1=-1.0, scalar2=None, op0=ALU.mult
        )

        # ------------- subtract lse, write output -------------
        for mi in range(16):
            for hf in range(2):
                W = HALF_W[hf]
                rl = spn.tile([128, 2048], DT.bfloat16, tag="span", bufs=2)
                nc.sync.dma_start(
                    in_=spill_d[mi, :, 2048 * hf : 2048 * hf + W], out=rl[:, :W]
                )
                oc = wk.tile([128, 2048], DT.float32, tag="xg")
                nc.scalar.activation(
                    out=oc[:, :W],
                    in_=rl[:, :W],
                    func=AF.Identity,
                    bias=nlse[:, mi : mi + 1],
                )
                for tl in range(2):
                    nc.sync.dma_start(
                        out=outp_d[:, 2 * mi + tl, 2048 * hf : 2048 * hf + W],
                        in_=oc[64 * tl : 64 * tl + 64, :W],
                    )

        for p in (dram, hsm, ps_pr, ps_gh, ps_a, spn, gt, wk, const):
            p.release()

    nc.compile()
    return nc


_NC_CACHE = {}


def _get_nc():
    if "nc" not in _NC_CACHE:
        _NC_CACHE["nc"] = build_nc()
    return _NC_CACHE["nc"]


def _tile_T(a):
    """[H, C] -> [128, KT*C] with element [p, C*k + c] = a[128k + p, c]."""
    Hd, C = a.shape
    k = Hd // 128
    return np.ascontiguousarray(a.reshape(k, 128, C).transpose(1, 0, 2).reshape(128, k * C))


def make_in_maps(
    encoder_outputs,
    encoder_hidden,
    input_mask,
    target_tensor,
    SOS_token,
    max_len,
    emb,
    W_ih,
    W_hh,
    b_ih,
    b_hh,
    out_W,
    out_b,
):
    emb = np.asarray(emb, np.float32)
    W_ih = np.asarray(W_ih, np.float32)
    W_hh = np.asarray(W_hh, np.float32)
    b_ih = np.asarray(b_ih, np.float32)
    b_hh = np.asarray(b_hh, np.float32)
    out_W = np.asarray(out_W, np.float32)
    out_b = np.asarray(out_b, np.float32)
    tgt = np.asarray(target_tensor)
    sos = int(SOS_token)

    tokens = np.concatenate(
        [np.full((B, 1), sos, dtype=np.int64), tgt[:, : T - 1]], axis=1
    ).astype(np.int32)  # [B, T]
    tok_tmaj = np.ascontiguousarray(tokens.T).reshape(R)  # r = t*64 + b
    tok_in = np.ascontiguousarray(tok_tmaj.reshape(16, 128).T)  # [128, 16]

    h0 = np.asarray(encoder_hidden, np.float32)[0]  # [64, 1024]
    h0T = np.ascontiguousarray(h0.T)  # [1024, 64]
    h0_hi = h0T.astype(BF16)
    h0_lo = (h0T - h0_hi.astype(np.float32)).astype(BF16)
    h0_pack = np.concatenate([h0_hi, h0_lo], axis=1)  # [1024, 128] (hi|lo)
    h0T_in = _tile_T(h0_pack)  # [128, 1024] bf16

    in_maps = []
    for j in range(NC):
        rows_j = np.concatenate(
            [g * H + j * 128 + np.arange(128) for g in range(3)]
        )  # [384] gate rows of this core
        whhT = _tile_T(np.ascontiguousarray(W_hh[rows_j].T))
        whh_in = whhT.astype(BF16)
        whl_in = (whhT - whh_in.astype(np.float32)).astype(BF16)
        wih_in = _tile_T(np.ascontiguousarray(W_ih[rows_j].T)).astype(np.float32)
        bhh_j = b_hh[rows_j].reshape(3, 128).T  # [128, 3]
        bih_j = b_ih[rows_j].reshape(3, 128).T
        bhh_in = np.ascontiguousarray(bhh_j * np.array([-1.0, -1.0, 1.0], np.float32))
        bih_in = np.ascontiguousarray(bih_j).astype(np.float32)
        woT_in = _tile_T(np.ascontiguousarray(out_W[j * VL : (j + 1) * VL].T)).astype(
            BF16
        )
        outb_in = np.ascontiguousarray(
            np.broadcast_to(out_b[j * VL : (j + 1) * VL], (128, VL))
        ).astype(BF16)
        h0sh_in = np.ascontiguousarray(h0T[j * 128 : (j + 1) * 128]).astype(np.float32)

        in_maps.append(
            {
                "tok": tok_in,
                "emb": emb,
                "h0T": h0T_in,
                "h0sh": h0sh_in,
                "whh": whh_in,
                "whl": whl_in,
                "wih": wih_in,
                "bhh": bhh_in,
                "bih": bih_in,
                "woT": woT_in,
                "outb": outb_in,
            }
        )
    return in_maps


def assemble(results):
    log_probs = np.concatenate(
        [results[j]["outp"] for j in range(NC)], axis=2
    ).astype(np.float32)
    h_final = np.ascontiguousarray(results[0]["hfin"].T)[None].astype(np.float32)
    return log_probs, h_final


def kernel(**inputs):
    from concourse.bass_utils import run_bass_kernel_spmd

    nc = _get_nc()
    in_maps = make_in_maps(**inputs)
    res = run_bass_kernel_spmd(nc, in_maps, core_ids=list(range(NC)))
    return assemble(res.results)


if __name__ == "__main__":
    # smoke-test the IR build only
    nc = build_nc()
    print("built OK; instructions:", sum(len(b.instructions) for b in nc.main_func.blocks))


# revision 20
# speedup vs baseline: 1.0748x; 1.0748x over previous
"""Trainium2 Bass/Tile kernel for nn_DecoderRNN (GRU decoder, teacher forcing,
vocab projection + log_softmax) on 8 NeuronCores.

Sharding:
  - GRU recurrence: hidden dim sharded 8-way (core j owns h columns
    [128j, 128j+128)); per-step AllGather (fp32, 32KB/rank) rebuilds the full
    hidden state on every core in transposed layout, which is exactly the
    stationary operand the recurrence and output projection need.
  - Output projection + log_softmax: vocab sharded 8-way (core j owns
    out_W rows [4000j, 4000j+4000)), interleaved step-by-step with the
    recurrence so the PE stays busy during collective latency.
  - Two AllReduces combine per-core sum(exp(logits)) into the global
    log_softmax denominator (rows split in half so output writes overlap
    the recurrence).

Layouts (device):
  rows r = t*64 + b (t-major). Recurrence/gi work batch-major
  ([64 rows, 384 gates] per step); W.T / x.T / hs.T tiles keep H on
  partitions as 8 k-tiles: [128p, k, cols]. Matmuls on fp32 data use the
  float32r bitcast (full-rate streaming for N>=256).
"""

import sys

sys.path.insert(0, "/opt/trn_rl_repo")

import numpy as np
import ml_dtypes

import concourse.bass as bass
import concourse.tile as tile
from concourse import bacc
from concourse import mybir
from concourse.bass import IndirectOffsetOnAxis
from concourse.masks import make_identity

AF = mybir.ActivationFunctionType
ALU = mybir.AluOpType
DT = mybir.dt
BF16 = ml_dtypes.bfloat16

B, S, H, V, T = 64, 32, 1024, 32000, 32
NC = 8
VL = V // NC  # 4000
KT = H // 128  # 8
R = T * B  # 2048
RG = [list(range(NC))]
HALF_W = (2048, 1952)  # projection half-span widths (4*512, 3*512+416)


def _v(t, k):
    """View a [128, k*c] tile as [128, k, c]."""
    return t[:].rearrange("p (k c) -> p k c", k=k)


def _r(ap):
    return ap.bitcast(DT.float32r)


def build_nc():
    nc = bacc.Bacc(
        "TRN2",
        target_bir_lowering=False,
        debug=False,
        num_devices=NC,
    )

    # ---------------- I/O ----------------
    tok_d = nc.dram_tensor("tok", [128, 16], DT.int32, kind="ExternalInput")
    emb_d = nc.dram_tensor("emb", [V, H], DT.float32, kind="ExternalInput")
    h0T_d = nc.dram_tensor("h0T", [128, KT * 64], DT.float32r, kind="ExternalInput")
    h0sh_d = nc.dram_tensor("h0sh", [64, 128], DT.float32, kind="ExternalInput")
    whh_d = nc.dram_tensor("whh", [128, KT * 384], DT.float32r, kind="ExternalInput")
    wih_d = nc.dram_tensor("wih", [128, KT * 384], DT.float32r, kind="ExternalInput")
    gib_d = nc.dram_tensor("gib", [128, 384], DT.float32, kind="ExternalInput")
    bhn_d = nc.dram_tensor("bhn", [128, 128], DT.float32, kind="ExternalInput")
    woT_d = nc.dram_tensor("woT", [128, KT * VL], DT.bfloat16, kind="ExternalInput")
    outb_d = nc.dram_tensor("outb", [128, VL], DT.bfloat16, kind="ExternalInput")

    outp_d = nc.dram_tensor("outp", [B, T, VL], DT.float32, kind="ExternalOutput")
    hfin_d = nc.dram_tensor("hfin", [H, B], DT.float32, kind="ExternalOutput")

    spill_d = nc.dram_tensor("spill", [16, 128, VL], DT.bfloat16)  # internal

    with tile.TileContext(nc) as tc:
        const = tc.alloc_tile_pool(name="const", bufs=1)
        wk = tc.alloc_tile_pool(name="wk", bufs=2)
        gt = tc.alloc_tile_pool(name="gt", bufs=2)
        spn = tc.alloc_tile_pool(name="spn", bufs=3)
        ps_a = tc.alloc_tile_pool(name="ps_a", bufs=2, space="PSUM")
        ps_gh = tc.alloc_tile_pool(name="ps_gh", bufs=2, space="PSUM")
        ps_pr = tc.alloc_tile_pool(name="ps_pr", bufs=1, space="PSUM")
        hsm = tc.alloc_tile_pool(name="hsm", bufs=4)
        dram = tc.alloc_tile_pool(name="dram", bufs=3, space="DRAM")

        # ------------- persistent SBUF -------------
        woT = const.tile([128, KT * VL], DT.bfloat16)
        whh = const.tile([128, KT * 384], DT.float32r)
        wih = const.tile([128, KT * 384], DT.float32r)
        h0T = const.tile([128, KT * 64], DT.float32r)
        outb = const.tile([128, VL], DT.bfloat16)
        gib = const.tile([128, 384], DT.float32)
        bhn = const.tile([128, 128], DT.float32)
        tok = const.tile([128, 16], DT.int32)
        acc = const.tile([128, 32], DT.float32)
        sloc = const.tile([128, 16], DT.float32)
        sglob = const.tile([128, 16], DT.float32)
        lse = const.tile([128, 16], DT.float32)
        nlse = const.tile([128, 16], DT.float32)
        h0sh = const.tile([64, 128], DT.float32)
        ident = const.tile([128, 128], DT.float32)

        # ------------- load constants -------------
        nc.sync.dma_start(out=woT[:], in_=woT_d[:, :])
        nc.sync.dma_start(out=whh[:], in_=whh_d[:, :])
        nc.sync.dma_start(out=wih[:], in_=wih_d[:, :])
        nc.sync.dma_start(out=outb[:], in_=outb_d[:, :])
        nc.sync.dma_start(out=gib[:], in_=gib_d[:, :])
        nc.sync.dma_start(out=bhn[:], in_=bhn_d[:, :])
        nc.sync.dma_start(out=tok[:], in_=tok_d[:, :])
        nc.sync.dma_start(out=h0T[:], in_=h0T_d[:, :])
        nc.sync.dma_start(out=h0sh[:], in_=h0sh_d[:, :])
        make_identity(nc, ident[:])

        whh_v = _v(whh, KT)  # [128, k, 384] f32
        wih_v = _v(wih, KT)
        woT_v = _v(woT, KT)  # [128, k, 4000] bf16
        h0T_v = _v(h0T, KT)  # [128, k, 64] f32

        xT_tiles = {}
        hs_tiles = {}
        gi_tiles = {}

        def emit_rblock(i):
            """Gather 128 embedding rows, relu+transpose into xT tile i."""
            xT_tiles[i] = wk.tile(
                [128, KT * 128], DT.float32r, name=f"xT{i}", tag="xT"
            )
            xch = _v(xT_tiles[i], KT)
            xg = wk.tile([128, H], DT.float32, tag="xg")
            nc.gpsimd.indirect_dma_start(
                out=xg[:],
                out_offset=None,
                in_=emb_d[:, :],
                in_offset=IndirectOffsetOnAxis(ap=tok[:, i : i + 1], axis=0),
            )
            for k in range(KT):
                pt = ps_a.tile([128, 128], DT.float32, tag="ps_a")
                nc.tensor.transpose(
                    out=pt[:, 0:128],
                    in_=xg[:, 128 * k : 128 * (k + 1)],
                    identity=ident[:],
                )
                # fused relu: x.T = max(x.T, 0)
                nc.vector.tensor_scalar(
                    out=xch[:, k, :],
                    in0=pt[:, 0:128],
                    scalar1=0.0,
                    scalar2=None,
                    op0=ALU.max,
                )

        def emit_gi(ts):
            """gi for step ts (rows [64*ts, 64*ts+64)) = x @ W_ih_j.T + biases."""
            mi, par = ts // 2, ts % 2
            if mi not in xT_tiles:
                emit_rblock(mi)
            xch = _v(xT_tiles[mi], KT)
            ps = ps_a.tile([64, 384], DT.float32, tag="ps_a")
            for k in range(KT):
                nc.tensor.matmul(
                    ps[:, :],
                    xch[:, k, 64 * par : 64 * par + 64],
                    wih_v[:, k, :],
                    start=(k == 0),
                    stop=(k == KT - 1),
                )
            gi_tiles[ts] = gt.tile(
                [64, 384], DT.float32, name=f"gi{ts}", tag="gi", bufs=5
            )
            # + b_ih (+ b_hh folded in for the r,z gate columns)
            nc.vector.tensor_tensor(
                out=gi_tiles[ts][:], in0=ps[:, :], in1=gib[0:64, :], op=ALU.add
            )
            if par == 1:
                del xT_tiles[mi]

        def emit_proj_half(mi, hf):
            """Projection for row-tile mi (rows 128mi..), vocab half hf."""
            W = HALF_W[hf]
            hsv = _v(hs_tiles[mi], KT)
            pp = ps_pr.tile([128, 2048], DT.float32, tag="ps_pr")
            for nl in range(4):
                n = 4 * hf + nl
                w = 512 if n < 7 else 416
                for k in range(KT):
                    nc.tensor.matmul(
                        pp[:, 512 * nl : 512 * nl + w],
                        hsv[:, k, 0:128],
                        woT_v[:, k, 512 * n : 512 * n + w],
                        start=(k == 0),
                        stop=(k == KT - 1),
                    )
            sp = spn.tile([128, 2048], DT.bfloat16, tag="span", bufs=2)
            nc.vector.tensor_tensor(
                out=sp[:, :W],
                in0=pp[:, :W],
                in1=outb[:, 2048 * hf : 2048 * hf + W],
                op=ALU.add,
            )
            ex = spn.tile([128, 2048], DT.bfloat16, tag="expo", bufs=1)
            nc.scalar.activation(
                out=ex[:, :W],
                in_=sp[:, :W],
                func=AF.Exp,
                accum_out=acc[:, 2 * mi + hf : 2 * mi + hf + 1],
            )
            nc.sync.dma_start(
                out=spill_d[mi, :, 2048 * hf : 2048 * hf + W], in_=sp[:, :W]
            )

        def emit_lse(half):
            """AllReduce sum(exp) for row-tiles [8*half, 8*half+8)."""
            lo = 8 * half
            for mi in range(lo, lo + 8):
                nc.vector.tensor_tensor(
                    out=sloc[:, mi : mi + 1],
                    in0=acc[:, 2 * mi : 2 * mi + 1],
                    in1=acc[:, 2 * mi + 1 : 2 * mi + 2],
                    op=ALU.add,
                )
            ar_i = dram.tile([128, 8], DT.float32, name=f"ar_i{half}", tag="ar_i")
            nc.sync.dma_start(out=ar_i[:], in_=sloc[:, lo : lo + 8])
            ar_o = dram.tile([128, 8], DT.float32, name=f"ar_o{half}", tag="ar_o")
            nc.gpsimd.collective_compute(
                "AllReduce",
                ALU.add,
                replica_groups=RG,
                ins=[ar_i[:].opt()],
                outs=[ar_o[:].opt()],
            )
            nc.sync.dma_start(out=sglob[:, lo : lo + 8], in_=ar_o[:])
            nc.scalar.activation(
                out=lse[:, lo : lo + 8], in_=sglob[:, lo : lo + 8], func=AF.Ln
            )
            nc.vector.tensor_scalar(
                out=nlse[:, lo : lo + 8],
                in0=lse[:, lo : lo + 8],
                scalar1=-1.0,
                scalar2=None,
                op0=ALU.mult,
            )

        def emit_out_chunk(mi, hf):
            """Reload spilled logits, subtract lse, write output rows."""
            W = HALF_W[hf]
            rl = spn.tile([128, 2048], DT.bfloat16, tag="span", bufs=2)
            nc.sync.dma_start(
                in_=spill_d[mi, :, 2048 * hf : 2048 * hf + W], out=rl[:, :W]
            )
            oc = wk.tile([128, 2048], DT.float32, tag="oc")
            nc.scalar.activation(
                out=oc[:, :W],
                in_=rl[:, :W],
                func=AF.Identity,
                bias=nlse[:, mi : mi + 1],
            )
            for tl in range(2):
                nc.sync.dma_start(
                    out=outp_d[:, 2 * mi + tl, 2048 * hf : 2048 * hf + W],
                    in_=oc[64 * tl : 64 * tl + 64, :W],
                )

        # ------------- prologue -------------
        emit_gi(0)
        emit_gi(1)

        # ------------- recurrence + interleaved projection -------------
        h_prev = h0sh  # [64, 128] f32, this core's h-column shard, batch-major
        hT_v = h0T_v  # [128, k, 64] f32 view of h_{t-1}.T

        for t in range(T):
            mi_t, par = t // 2, t % 2

            # gh = h_{t-1} @ W_hh_j.T  (batch-major, fp32r full-rate)
            gh = ps_gh.tile([64, 384], DT.float32, tag="ps_gh")
            for k in range(KT):
                nc.tensor.matmul(
                    gh[:, :],
                    hT_v[:, k, :],
                    whh_v[:, k, :],
                    start=(k == 0),
                    stop=(k == KT - 1),
                )

            gi_t = gi_tiles[t][:]  # [64, 384]

            # r = sigma(gh_r + gi_r + b), z likewise; sigma via exp+recip
            tr = gt.tile([64, 128], DT.float32, tag="tr")
            nc.vector.tensor_tensor(
                out=tr[:], in0=gh[:, 0:128], in1=gi_t[:, 0:128], op=ALU.add
            )
            er = gt.tile([64, 128], DT.float32, tag="er")
            nc.scalar.activation(out=er[:], in_=tr[:], func=AF.Exp, scale=-1.0)
            dr = gt.tile([64, 128], DT.float32, tag="dr")
            nc.vector.tensor_scalar(
                out=dr[:], in0=er[:], scalar1=1.0, scalar2=None, op0=ALU.add
            )
            rr = gt.tile([64, 128], DT.float32, tag="rr")
            nc.vector.reciprocal(out=rr[:], in_=dr[:])

            tz = gt.tile([64, 128], DT.float32, tag="tz")
            nc.vector.tensor_tensor(
                out=tz[:], in0=gh[:, 128:256], in1=gi_t[:, 128:256], op=ALU.add
            )
            ez = gt.tile([64, 128], DT.float32, tag="ez")
            nc.scalar.activation(out=ez[:], in_=tz[:], func=AF.Exp, scale=-1.0)
            dz = gt.tile([64, 128], DT.float32, tag="dz")
            nc.vector.tensor_scalar(
                out=dz[:], in0=ez[:], scalar1=1.0, scalar2=None, op0=ALU.add
            )
            zz = gt.tile([64, 128], DT.float32, tag="zz")
            nc.vector.reciprocal(out=zz[:], in_=dz[:])

            # n = tanh(gi_n + r*(gh_n + b_hh_n)); tanh via exp+recip
            hn = gt.tile([64, 128], DT.float32, tag="hn")
            nc.vector.tensor_tensor(
                out=hn[:], in0=gh[:, 256:384], in1=bhn[0:64, :], op=ALU.add
            )
            uu = gt.tile([64, 128], DT.float32, tag="uu")
            nc.vector.tensor_tensor(out=uu[:], in0=rr[:], in1=hn[:], op=ALU.mult)
            ww = gt.tile([64, 128], DT.float32, tag="ww")
            nc.vector.tensor_tensor(
                out=ww[:], in0=uu[:], in1=gi_t[:, 256:384], op=ALU.add
            )
            en = gt.tile([64, 128], DT.float32, tag="en")
            nc.scalar.activation(out=en[:], in_=ww[:], func=AF.Exp, scale=-2.0)
            dn = gt.tile([64, 128], DT.float32, tag="dn")
            nc.vector.tensor_scalar(
                out=dn[:], in0=en[:], scalar1=1.0, scalar2=None, op0=ALU.add
            )
            pn = gt.tile([64, 128], DT.float32, tag="pn")
            nc.vector.reciprocal(out=pn[:], in_=dn[:])
            nn = gt.tile([64, 128], DT.float32, tag="nn")
            nc.vector.tensor_scalar(
                out=nn[:],
                in0=pn[:],
                scalar1=2.0,
                scalar2=-1.0,
                op0=ALU.mult,
                op1=ALU.add,
            )

            # h' = n + z*(h - n)
            d2 = gt.tile([64, 128], DT.float32, tag="d2")
            nc.vector.tensor_tensor(
                out=d2[:], in0=h_prev[:], in1=nn[:], op=ALU.subtract
            )
            qq = gt.tile([64, 128], DT.float32, tag="qq")
            nc.vector.tensor_tensor(out=qq[:], in0=zz[:], in1=d2[:], op=ALU.mult)
            hnew = gt.tile([64, 128], DT.float32, tag="hnew")
            nc.vector.tensor_tensor(out=hnew[:], in0=nn[:], in1=qq[:], op=ALU.add)

            # transpose h' -> [128, 64] and AllGather across cores
            pt = ps_a.tile([128, 64], DT.float32, tag="ps_a")
            nc.tensor.transpose(
                out=pt[:, 0:64], in_=hnew[:], identity=ident[0:64, 0:64]
            )
            htf = gt.tile([128, 64], DT.float32r, tag="htf")
            nc.vector.tensor_copy(out=htf[:], in_=pt[:, 0:64])

            agi = dram.tile([128, 64], DT.float32r, tag="agi")
            nc.sync.dma_start(out=agi[:], in_=htf[:])
            ago = dram.tile([KT * 128, 64], DT.float32r, tag="ago")
            nc.gpsimd.collective_compute(
                "AllGather",
                ALU.bypass,
                replica_groups=RG,
                ins=[agi[:].opt()],
                outs=[ago[:].opt()],
            )
            ago_v = ago[:].rearrange("(k p) b -> p k b", p=128)

            hcur = gt.tile([128, KT * 64], DT.float32r, tag="hcur")
            nc.sync.dma_start(out=_v(hcur, KT), in_=ago_v)
            if par == 0:
                hs_tiles[mi_t] = hsm.tile(
                    [128, KT * 128], DT.bfloat16, name=f"hs{mi_t}", tag="hsm"
                )
            hsv_t = _v(hs_tiles[mi_t], KT)
            # cast fp32 -> bf16 during DMA (SWDGE)
            nc.gpsimd.dma_start(
                out=hsv_t[:, :, 64 * par : 64 * par + 64],
                in_=ago_v.bitcast(DT.float32),
            )
            if t == T - 1:
                nc.gpsimd.dma_start(out=hfin_d[:, :], in_=ago[:].bitcast(DT.float32))

            h_prev = hnew
            hT_v = _v(hcur, KT)

            del gi_tiles[t]
            # trailing prologue work
            if t < 30:
                emit_gi(t + 2)
            # interleaved projection halves
            if t >= 2:
                emit_proj_half((t - 2) // 2, (t - 2) % 2)
            # first-half lse + interleaved output writes
            if t == 17:
                emit_lse(0)
            if 18 <= t <= 25:
                emit_out_chunk(t - 18, 0)
                emit_out_chunk(t - 18, 1)

        # remaining projection halves
        emit_proj_half(15, 0)
        emit_proj_half(15, 1)

        # ------------- second-half lse + remaining output -------------
        emit_lse(1)
        for mi in range(8, 16):
            emit_out_chunk(mi, 0)
            emit_out_chunk(mi, 1)

        for p in (dram, hsm, ps_pr, ps_gh, ps_a, spn, gt, wk, const):
            p.release()

    nc.compile()
    return nc


_NC_CACHE = {}


def _get_nc():
    if "nc" not in _NC_CACHE:
        _NC_CACHE["nc"] = build_nc()
    return _NC_CACHE["nc"]


def _tile_T(a):
    """[H, C] -> [128, KT*C] with element [p, C*k + c] = a[128k + p, c]."""
    Hd, C = a.shape
    k = Hd // 128
    return np.ascontiguousarray(
        a.reshape(k, 128, C).transpose(1, 0, 2).reshape(128, k * C)
    )


def make_in_maps(
    encoder_outputs,
    encoder_hidden,
    input_mask,
    target_tensor,
    SOS_token,
    max_len,
    emb,
    W_ih,
    W_hh,
    b_ih,
    b_hh,
    out_W,
    out_b,
):
    emb = np.asarray(emb, np.float32)
    W_ih = np.asarray(W_ih, np.float32)
    W_hh = np.asarray(W_hh, np.float32)
    b_ih = np.asarray(b_ih, np.float32)
    b_hh = np.asarray(b_hh, np.float32)
    out_W = np.asarray(out_W, np.float32)
    out_b = np.asarray(out_b, np.float32)
    tgt = np.asarray(target_tensor)
    sos = int(SOS_token)

    tokens = np.concatenate(
        [np.full((B, 1), sos, dtype=np.int64), tgt[:, : T - 1]], axis=1
    ).astype(np.int32)  # [B, T]
    tok_tmaj = np.ascontiguousarray(tokens.T).reshape(R)  # r = t*64 + b
    tok_in = np.ascontiguousarray(tok_tmaj.reshape(16, 128).T)  # [128, 16]

    h0 = np.asarray(encoder_hidden, np.float32)[0]  # [64, 1024]
    h0T = np.ascontiguousarray(h0.T)  # [1024, 64]
    h0T_in = _tile_T(h0T).astype(np.float32)  # [128, 512]

    in_maps = []
    for j in range(NC):
        rows_j = np.concatenate(
            [g * H + j * 128 + np.arange(128) for g in range(3)]
        )  # [384] gate rows of this core
        whh_in = _tile_T(np.ascontiguousarray(W_hh[rows_j].T)).astype(np.float32)
        wih_in = _tile_T(np.ascontiguousarray(W_ih[rows_j].T)).astype(np.float32)
        # gi bias = b_ih (+ b_hh folded in for the r,z gate columns)
        gib_row = b_ih[rows_j].copy()
        gib_row[0:256] += b_hh[rows_j[0:256]]
        gib_in = np.ascontiguousarray(
            np.broadcast_to(gib_row.astype(np.float32), (128, 384))
        )
        bhn_in = np.ascontiguousarray(
            np.broadcast_to(b_hh[rows_j[256:384]].astype(np.float32), (128, 128))
        )
        woT_in = _tile_T(
            np.ascontiguousarray(out_W[j * VL : (j + 1) * VL].T)
        ).astype(BF16)
        outb_in = np.ascontiguousarray(
            np.broadcast_to(out_b[j * VL : (j + 1) * VL], (128, VL))
        ).astype(BF16)
        h0sh_in = np.ascontiguousarray(h0[:, j * 128 : (j + 1) * 128]).astype(
            np.float32
        )

        in_maps.append(
            {
                "tok": tok_in,
                "emb": emb,
                "h0T": h0T_in,
                "h0sh": h0sh_in,
                "whh": whh_in,
                "wih": wih_in,
                "gib": gib_in,
                "bhn": bhn_in,
                "woT": woT_in,
                "outb": outb_in,
            }
        )
    return in_maps


def assemble(results):
    log_probs = np.concatenate(
        [results[j]["outp"] for j in range(NC)], axis=2
    ).astype(np.float32)
    h_final = np.ascontiguousarray(results[0]["hfin"].T)[None].astype(np.float32)
    return log_probs, h_final


def kernel(**inputs):
    from concourse.bass_utils import run_bass_kernel_spmd

    nc = _get_nc()
    in_maps = make_in_maps(**inputs)
    res = run_bass_kernel_spmd(nc, in_maps, core_ids=list(range(NC)))
    return assemble(res.results)


if __name__ == "__main__":
    nc = build_nc()
    print(
        "built OK; instructions:",
        sum(len(b.instructions) for b in nc.main_func.blocks),
    )


# revision 21
# speedup vs baseline: 1.1400x; 1.0607x over previous
"""Trainium2 Bass/Tile kernel for nn_DecoderRNN (GRU decoder, teacher forcing,
vocab projection + log_softmax) on 8 NeuronCores.

Sharding:
  - GRU recurrence: hidden dim sharded 8-way (core j owns h columns
    [128j, 128j+128)); per-step AllGather (fp32, 32KB/rank) rebuilds the full
    hidden state on every core in transposed layout, which is exactly the
    stationary operand the recurrence and output projection need.
  - Output projection + log_softmax: vocab sharded 8-way (core j owns
    out_W rows [4000j, 4000j+4000)), interleaved step-by-step with the
    recurrence so the PE stays busy during collective latency.
  - Two AllReduces combine per-core sum(exp(logits)) into the global
    log_softmax denominator (rows split in half so output writes overlap
    the recurrence).

Layouts (device):
  rows r = t*64 + b (t-major). Recurrence/gi work batch-major
  ([64 rows, 384 gates] per step); W.T / x.T / hs.T tiles keep H on
  partitions as 8 k-tiles: [128p, k, cols]. Matmuls on fp32 data use the
  float32r bitcast (full-rate streaming for N>=256).
"""

import sys

sys.path.insert(0, "/opt/trn_rl_repo")

import numpy as np
import ml_dtypes

import concourse.bass as bass
import concourse.tile as tile
from concourse import bacc
from concourse import mybir
from concourse.bass import IndirectOffsetOnAxis
from concourse.masks import make_identity

AF = mybir.ActivationFunctionType
ALU = mybir.AluOpType
DT = mybir.dt
BF16 = ml_dtypes.bfloat16

B, S, H, V, T = 64, 32, 1024, 32000, 32
NC = 8
VL = V // NC  # 4000
KT = H // 128  # 8
R = T * B  # 2048
RG = [list(range(NC))]
HALF_W = (2048, 1952)  # projection half-span widths (4*512, 3*512+416)


def _v(t, k):
    """View a [128, k*c] tile as [128, k, c]."""
    return t[:].rearrange("p (k c) -> p k c", k=k)


def _r(ap):
    return ap.bitcast(DT.float32r)


def build_nc():
    nc = bacc.Bacc(
        "TRN2",
        target_bir_lowering=False,
        debug=False,
        num_devices=NC,
    )

    # ---------------- I/O ----------------
    tok_d = nc.dram_tensor("tok", [128, 16], DT.int32, kind="ExternalInput")
    emb_d = nc.dram_tensor("emb", [V, H], DT.float32, kind="ExternalInput")
    h0T_d = nc.dram_tensor("h0T", [128, KT * 64], DT.float32r, kind="ExternalInput")
    h0sh_d = nc.dram_tensor("h0sh", [64, 128], DT.float32, kind="ExternalInput")
    whh_d = nc.dram_tensor("whh", [128, KT * 384], DT.float32r, kind="ExternalInput")
    wih_d = nc.dram_tensor("wih", [128, KT * 384], DT.float32r, kind="ExternalInput")
    gib_d = nc.dram_tensor("gib", [128, 384], DT.float32, kind="ExternalInput")
    bhn_d = nc.dram_tensor("bhn", [128, 128], DT.float32r, kind="ExternalInput")
    idr_d = nc.dram_tensor("idr", [64, 64], DT.float32r, kind="ExternalInput")
    woT_d = nc.dram_tensor("woT", [128, KT * VL], DT.bfloat16, kind="ExternalInput")
    outb_d = nc.dram_tensor("outb", [128, VL], DT.bfloat16, kind="ExternalInput")

    outp_d = nc.dram_tensor("outp", [B, T, VL], DT.float32, kind="ExternalOutput")
    hfin_d = nc.dram_tensor("hfin", [H, B], DT.float32, kind="ExternalOutput")

    spill_d = nc.dram_tensor("spill", [16, 128, VL], DT.bfloat16)  # internal

    with tile.TileContext(nc) as tc:
        const = tc.alloc_tile_pool(name="const", bufs=1)
        wk = tc.alloc_tile_pool(name="wk", bufs=2)
        gt = tc.alloc_tile_pool(name="gt", bufs=2)
        spn = tc.alloc_tile_pool(name="spn", bufs=3)
        ps_a = tc.alloc_tile_pool(name="ps_a", bufs=2, space="PSUM")
        ps_gh = tc.alloc_tile_pool(name="ps_gh", bufs=2, space="PSUM")
        ps_pr = tc.alloc_tile_pool(name="ps_pr", bufs=1, space="PSUM")
        hsm = tc.alloc_tile_pool(name="hsm", bufs=4)
        dram = tc.alloc_tile_pool(name="dram", bufs=3, space="DRAM")

        # ------------- persistent SBUF -------------
        woT = const.tile([128, KT * VL], DT.bfloat16)
        whh = const.tile([128, KT * 384], DT.float32r)
        wih = const.tile([128, KT * 384], DT.float32r)
        h0T = const.tile([128, KT * 64], DT.float32r)
        outb = const.tile([128, VL], DT.bfloat16)
        gib = const.tile([128, 384], DT.float32)
        bhn = const.tile([128, 128], DT.float32r)
        idr = const.tile([64, 64], DT.float32r)
        tok = const.tile([128, 16], DT.int32)
        acc = const.tile([128, 32], DT.float32)
        sloc = const.tile([128, 16], DT.float32)
        sglob = const.tile([128, 16], DT.float32)
        lse = const.tile([128, 16], DT.float32)
        nlse = const.tile([128, 16], DT.float32)
        h0sh = const.tile([64, 128], DT.float32)
        ident = const.tile([128, 128], DT.float32)

        # ------------- load constants -------------
        nc.sync.dma_start(out=woT[:], in_=woT_d[:, :])
        nc.sync.dma_start(out=whh[:], in_=whh_d[:, :])
        nc.sync.dma_start(out=wih[:], in_=wih_d[:, :])
        nc.sync.dma_start(out=outb[:], in_=outb_d[:, :])
        nc.sync.dma_start(out=gib[:], in_=gib_d[:, :])
        nc.sync.dma_start(out=bhn[:], in_=bhn_d[:, :])
        nc.sync.dma_start(out=idr[:], in_=idr_d[:, :])
        nc.sync.dma_start(out=tok[:], in_=tok_d[:, :])
        nc.sync.dma_start(out=h0T[:], in_=h0T_d[:, :])
        nc.sync.dma_start(out=h0sh[:], in_=h0sh_d[:, :])
        make_identity(nc, ident[:])

        whh_v = _v(whh, KT)  # [128, k, 384] f32
        wih_v = _v(wih, KT)
        woT_v = _v(woT, KT)  # [128, k, 4000] bf16
        h0T_v = _v(h0T, KT)  # [128, k, 64] f32

        xT_tiles = {}
        hs_tiles = {}
        gi_tiles = {}

        def emit_rblock(i):
            """Gather 128 embedding rows, relu+transpose into xT tile i."""
            xT_tiles[i] = wk.tile(
                [128, KT * 128], DT.float32r, name=f"xT{i}", tag="xT"
            )
            xch = _v(xT_tiles[i], KT)
            xg = wk.tile([128, H], DT.float32, tag="xg")
            nc.gpsimd.indirect_dma_start(
                out=xg[:],
                out_offset=None,
                in_=emb_d[:, :],
                in_offset=IndirectOffsetOnAxis(ap=tok[:, i : i + 1], axis=0),
            )
            for k in range(KT):
                pt = ps_a.tile([128, 128], DT.float32, tag="ps_a")
                nc.tensor.transpose(
                    out=pt[:, 0:128],
                    in_=xg[:, 128 * k : 128 * (k + 1)],
                    identity=ident[:],
                )
                # fused relu: x.T = max(x.T, 0)
                nc.vector.tensor_scalar(
                    out=xch[:, k, :],
                    in0=pt[:, 0:128],
                    scalar1=0.0,
                    scalar2=None,
                    op0=ALU.max,
                )

        def emit_gi(ts):
            """gi for step ts (rows [64*ts, 64*ts+64)) = x @ W_ih_j.T + biases."""
            mi, par = ts // 2, ts % 2
            if mi not in xT_tiles:
                emit_rblock(mi)
            xch = _v(xT_tiles[mi], KT)
            ps = ps_a.tile([64, 384], DT.float32, tag="ps_a")
            for k in range(KT):
                nc.tensor.matmul(
                    ps[:, :],
                    xch[:, k, 64 * par : 64 * par + 64],
                    wih_v[:, k, :],
                    start=(k == 0),
                    stop=(k == KT - 1),
                )
            gi_tiles[ts] = gt.tile(
                [64, 384], DT.float32r, name=f"gi{ts}", tag="gi", bufs=5
            )
            # + b_ih (+ b_hh folded in for the r,z gate columns)
            nc.vector.tensor_tensor(
                out=gi_tiles[ts][:], in0=ps[:, :], in1=gib[0:64, :], op=ALU.add
            )
            if par == 1:
                del xT_tiles[mi]

        def emit_proj_half(mi, hf):
            """Projection for row-tile mi (rows 128mi..), vocab half hf."""
            W = HALF_W[hf]
            hsv = _v(hs_tiles[mi], KT)
            pp = ps_pr.tile([128, 2048], DT.float32, tag="ps_pr")
            for nl in range(4):
                n = 4 * hf + nl
                w = 512 if n < 7 else 416
                for k in range(KT):
                    nc.tensor.matmul(
                        pp[:, 512 * nl : 512 * nl + w],
                        hsv[:, k, 0:128],
                        woT_v[:, k, 512 * n : 512 * n + w],
                        start=(k == 0),
                        stop=(k == KT - 1),
                    )
            sp = spn.tile([128, 2048], DT.bfloat16, tag="span", bufs=2)
            nc.vector.tensor_tensor(
                out=sp[:, :W],
                in0=pp[:, :W],
                in1=outb[:, 2048 * hf : 2048 * hf + W],
                op=ALU.add,
            )
            ex = spn.tile([128, 2048], DT.bfloat16, tag="expo", bufs=1)
            nc.scalar.activation(
                out=ex[:, :W],
                in_=sp[:, :W],
                func=AF.Exp,
                accum_out=acc[:, 2 * mi + hf : 2 * mi + hf + 1],
            )
            nc.sync.dma_start(
                out=spill_d[mi, :, 2048 * hf : 2048 * hf + W], in_=sp[:, :W]
            )

        def emit_lse(half):
            """AllReduce sum(exp) for row-tiles [8*half, 8*half+8)."""
            lo = 8 * half
            for mi in range(lo, lo + 8):
                nc.vector.tensor_tensor(
                    out=sloc[:, mi : mi + 1],
                    in0=acc[:, 2 * mi : 2 * mi + 1],
                    in1=acc[:, 2 * mi + 1 : 2 * mi + 2],
                    op=ALU.add,
                )
            ar_i = dram.tile([128, 8], DT.float32, name=f"ar_i{half}", tag="ar_i")
            nc.sync.dma_start(out=ar_i[:], in_=sloc[:, lo : lo + 8])
            ar_o = dram.tile([128, 8], DT.float32, name=f"ar_o{half}", tag="ar_o")
            nc.gpsimd.collective_compute(
                "AllReduce",
                ALU.add,
                replica_groups=RG,
                ins=[ar_i[:].opt()],
                outs=[ar_o[:].opt()],
            )
            nc.sync.dma_start(out=sglob[:, lo : lo + 8], in_=ar_o[:])
            nc.scalar.activation(
                out=lse[:, lo : lo + 8], in_=sglob[:, lo : lo + 8], func=AF.Ln
            )
            nc.vector.tensor_scalar(
                out=nlse[:, lo : lo + 8],
                in0=lse[:, lo : lo + 8],
                scalar1=-1.0,
                scalar2=None,
                op0=ALU.mult,
            )

        def emit_out_chunk(mi, hf):
            """Reload spilled logits, subtract lse, write output rows."""
            W = HALF_W[hf]
            rl = spn.tile([128, 2048], DT.bfloat16, tag="span", bufs=2)
            nc.sync.dma_start(
                in_=spill_d[mi, :, 2048 * hf : 2048 * hf + W], out=rl[:, :W]
            )
            oc = wk.tile([128, 2048], DT.float32, tag="oc")
            nc.scalar.activation(
                out=oc[:, :W],
                in_=rl[:, :W],
                func=AF.Identity,
                bias=nlse[:, mi : mi + 1],
            )
            for tl in range(2):
                nc.sync.dma_start(
                    out=outp_d[:, 2 * mi + tl, 2048 * hf : 2048 * hf + W],
                    in_=oc[64 * tl : 64 * tl + 64, :W],
                )

        # ------------- prologue -------------
        emit_gi(0)
        emit_gi(1)

        # ------------- recurrence + interleaved projection -------------
        h_prev = h0sh  # [64, 128] f32, this core's h-column shard, batch-major
        hT_v = h0T_v  # [128, k, 64] f32 view of h_{t-1}.T

        for t in range(T):
            mi_t, par = t // 2, t % 2

            # gh = h_{t-1} @ W_hh_j.T  (batch-major, fp32r full-rate)
            gh = ps_gh.tile([64, 384], DT.float32, tag="ps_gh")
            for k in range(KT):
                nc.tensor.matmul(
                    gh[:, :],
                    hT_v[:, k, :],
                    whh_v[:, k, :],
                    start=(k == 0),
                    stop=False,
                )

            gi_t = gi_tiles[t][:]  # [64, 384] float32r

            # fold gi (r,z) and b_hh_n into the gate psum on the PE
            nc.tensor.matmul(
                gh[:, 0:256], idr[:], gi_t[:, 0:256], start=False, stop=False
            )
            nc.tensor.matmul(
                gh[:, 256:384], idr[:], bhn[0:64, :], start=False, stop=True
            )

            # r = sigma(gh_r), z = sigma(gh_z); sigma via exp + fast recip
            er = gt.tile([64, 128], DT.float32, tag="er")
            nc.scalar.activation(out=er[:], in_=gh[:, 0:128], func=AF.Exp, scale=-1.0)
            dr = gt.tile([64, 128], DT.float32, tag="dr")
            nc.vector.tensor_scalar(
                out=dr[:], in0=er[:], scalar1=1.0, scalar2=None, op0=ALU.add
            )
            rr = gt.tile([64, 128], DT.float32, tag="rr")
            nc.vector.reciprocal_approx_fast(out=rr[:], in_=dr[:])

            ez = gt.tile([64, 128], DT.float32, tag="ez")
            nc.scalar.activation(out=ez[:], in_=gh[:, 128:256], func=AF.Exp, scale=-1.0)
            dz = gt.tile([64, 128], DT.float32, tag="dz")
            nc.vector.tensor_scalar(
                out=dz[:], in0=ez[:], scalar1=1.0, scalar2=None, op0=ALU.add
            )
            zz = gt.tile([64, 128], DT.float32, tag="zz")
            nc.vector.reciprocal_approx_fast(out=zz[:], in_=dz[:])

            # n-branch: n = tanh(gi_n + r*ghn'), ghn' = gh_n + b_hh_n (in psum)
            uu = gt.tile([64, 128], DT.float32, tag="uu")
            nc.vector.tensor_tensor(
                out=uu[:], in0=rr[:], in1=gh[:, 256:384], op=ALU.mult
            )
            ww = gt.tile([64, 128], DT.float32, tag="ww")
            nc.vector.tensor_tensor(
                out=ww[:],
                in0=uu[:],
                in1=gi_t[:, 256:384].bitcast(DT.float32),
                op=ALU.add,
            )
            en = gt.tile([64, 128], DT.float32, tag="en")
            nc.scalar.activation(out=en[:], in_=ww[:], func=AF.Exp, scale=-2.0)
            dn = gt.tile([64, 128], DT.float32, tag="dn")
            nc.vector.tensor_scalar(
                out=dn[:], in0=en[:], scalar1=1.0, scalar2=None, op0=ALU.add
            )
            pn = gt.tile([64, 128], DT.float32, tag="pn")
            nc.vector.reciprocal_approx_fast(out=pn[:], in_=dn[:])

            # h' = n + z*(h-n) with n = 2p-1:
            #   h' = 2*(1-z)*p + (z*h - (1-z))   [tail = 2 ops after p]
            u1 = gt.tile([64, 128], DT.float32, tag="u1")
            nc.vector.tensor_tensor(
                out=u1[:], in0=zz[:], in1=h_prev[:], op=ALU.mult
            )
            w1 = gt.tile([64, 128], DT.float32, tag="w1")
            nc.vector.tensor_scalar(
                out=w1[:], in0=zz[:], scalar1=-1.0, scalar2=1.0,
                op0=ALU.mult, op1=ALU.add,
            )
            c1 = gt.tile([64, 128], DT.float32, tag="c1")
            nc.vector.tensor_tensor(
                out=c1[:], in0=u1[:], in1=w1[:], op=ALU.subtract
            )
            vv = gt.tile([64, 128], DT.float32, tag="vv")
            nc.vector.tensor_tensor(out=vv[:], in0=w1[:], in1=pn[:], op=ALU.mult)
            hnew = gt.tile([64, 128], DT.float32, tag="hnew")
            nc.vector.scalar_tensor_tensor(
                out=hnew[:], in0=vv[:], scalar=2.0, in1=c1[:],
                op0=ALU.mult, op1=ALU.add,
            )

            # transpose h' -> [128, 64] and AllGather across cores
            pt = ps_a.tile([128, 64], DT.float32, tag="ps_a")
            nc.tensor.transpose(
                out=pt[:, 0:64], in_=hnew[:], identity=ident[0:64, 0:64]
            )
            htf = gt.tile([128, 64], DT.float32r, tag="htf")
            nc.vector.tensor_copy(out=htf[:], in_=pt[:, 0:64])

            agi = dram.tile([128, 64], DT.float32r, tag="agi")
            nc.sync.dma_start(out=agi[:], in_=htf[:])
            ago = dram.tile([KT * 128, 64], DT.float32r, tag="ago")
            nc.gpsimd.collective_compute(
                "AllGather",
                ALU.bypass,
                replica_groups=RG,
                ins=[agi[:].opt()],
                outs=[ago[:].opt()],
            )
            ago_v = ago[:].rearrange("(k p) b -> p k b", p=128)

            hcur = gt.tile([128, KT * 64], DT.float32r, tag="hcur")
            nc.sync.dma_start(out=_v(hcur, KT), in_=ago_v)
            if par == 0:
                hs_tiles[mi_t] = hsm.tile(
                    [128, KT * 128], DT.bfloat16, name=f"hs{mi_t}", tag="hsm"
                )
            hsv_t = _v(hs_tiles[mi_t], KT)
            # cast fp32 -> bf16 during DMA (SWDGE)
            nc.gpsimd.dma_start(
                out=hsv_t[:, :, 64 * par : 64 * par + 64],
                in_=ago_v.bitcast(DT.float32),
            )
            if t == T - 1:
                nc.gpsimd.dma_start(out=hfin_d[:, :], in_=ago[:].bitcast(DT.float32))

            h_prev = hnew
            hT_v = _v(hcur, KT)

            del gi_tiles[t]
            # trailing prologue work
            if t < 30:
                emit_gi(t + 2)
            # interleaved projection halves
            if t >= 2:
                emit_proj_half((t - 2) // 2, (t - 2) % 2)
            # first-half lse + interleaved output writes
            if t == 17:
                emit_lse(0)
            if 18 <= t <= 25:
                emit_out_chunk(t - 18, 0)
                emit_out_chunk(t - 18, 1)

        # remaining projection halves
        emit_proj_half(15, 0)
        emit_proj_half(15, 1)

        # ------------- second-half lse + remaining output -------------
        emit_lse(1)
        for mi in range(8, 16):
            emit_out_chunk(mi, 0)
            emit_out_chunk(mi, 1)

        for p in (dram, hsm, ps_pr, ps_gh, ps_a, spn, gt, wk, const):
            p.release()

    nc.compile()
    return nc


_NC_CACHE = {}


def _get_nc():
    if "nc" not in _NC_CACHE:
        _NC_CACHE["nc"] = build_nc()
    return _NC_CACHE["nc"]


def _tile_T(a):
    """[H, C] -> [128, KT*C] with element [p, C*k + c] = a[128k + p, c]."""
    Hd, C = a.shape
    k = Hd // 128
    return np.ascontiguousarray(
        a.reshape(k, 128, C).transpose(1, 0, 2).reshape(128, k * C)
    )


def make_in_maps(
    encoder_outputs,
    encoder_hidden,
    input_mask,
    target_tensor,
    SOS_token,
    max_len,
    emb,
    W_ih,
    W_hh,
    b_ih,
    b_hh,
    out_W,
    out_b,
):
    emb = np.asarray(emb, np.float32)
    W_ih = np.asarray(W_ih, np.float32)
    W_hh = np.asarray(W_hh, np.float32)
    b_ih = np.asarray(b_ih, np.float32)
    b_hh = np.asarray(b_hh, np.float32)
    out_W = np.asarray(out_W, np.float32)
    out_b = np.asarray(out_b, np.float32)
    tgt = np.asarray(target_tensor)
    sos = int(SOS_token)

    tokens = np.concatenate(
        [np.full((B, 1), sos, dtype=np.int64), tgt[:, : T - 1]], axis=1
    ).astype(np.int32)  # [B, T]
    tok_tmaj = np.ascontiguousarray(tokens.T).reshape(R)  # r = t*64 + b
    tok_in = np.ascontiguousarray(tok_tmaj.reshape(16, 128).T)  # [128, 16]

    h0 = np.asarray(encoder_hidden, np.float32)[0]  # [64, 1024]
    h0T = np.ascontiguousarray(h0.T)  # [1024, 64]
    h0T_in = _tile_T(h0T).astype(np.float32)  # [128, 512]

    in_maps = []
    for j in range(NC):
        rows_j = np.concatenate(
            [g * H + j * 128 + np.arange(128) for g in range(3)]
        )  # [384] gate rows of this core
        whh_in = _tile_T(np.ascontiguousarray(W_hh[rows_j].T)).astype(np.float32)
        wih_in = _tile_T(np.ascontiguousarray(W_ih[rows_j].T)).astype(np.float32)
        # gi bias = b_ih (+ b_hh folded in for the r,z gate columns)
        gib_row = b_ih[rows_j].copy()
        gib_row[0:256] += b_hh[rows_j[0:256]]
        gib_in = np.ascontiguousarray(
            np.broadcast_to(gib_row.astype(np.float32), (128, 384))
        )
        bhn_in = np.ascontiguousarray(
            np.broadcast_to(b_hh[rows_j[256:384]].astype(np.float32), (128, 128))
        )
        woT_in = _tile_T(
            np.ascontiguousarray(out_W[j * VL : (j + 1) * VL].T)
        ).astype(BF16)
        outb_in = np.ascontiguousarray(
            np.broadcast_to(out_b[j * VL : (j + 1) * VL], (128, VL))
        ).astype(BF16)
        h0sh_in = np.ascontiguousarray(h0[:, j * 128 : (j + 1) * 128]).astype(
            np.float32
        )

        in_maps.append(
            {
                "idr": np.eye(64, dtype=np.float32),
                "tok": tok_in,
                "emb": emb,
                "h0T": h0T_in,
                "h0sh": h0sh_in,
                "whh": whh_in,
                "wih": wih_in,
                "gib": gib_in,
                "bhn": bhn_in,
                "woT": woT_in,
                "outb": outb_in,
            }
        )
    return in_maps


def assemble(results):
    log_probs = np.concatenate(
        [results[j]["outp"] for j in range(NC)], axis=2
    ).astype(np.float32)
    h_final = np.ascontiguousarray(results[0]["hfin"].T)[None].astype(np.float32)
    return log_probs, h_final


def kernel(**inputs):
    from concourse.bass_utils import run_bass_kernel_spmd

    nc = _get_nc()
    in_maps = make_in_maps(**inputs)
    res = run_bass_kernel_spmd(nc, in_maps, core_ids=list(range(NC)))
    return assemble(res.results)


if __name__ == "__main__":
    nc = build_nc()
    print(
        "built OK; instructions:",
        sum(len(b.instructions) for b in nc.main_func.blocks),
    )
